# revision 11
# baseline (speedup 1.0000x reference)
"""Cross-attention kernel for Trainium2, SPMD over 8 NeuronCores.

Problem: B=4, N=2048, C=1024 fp32.
  q = event_f @ Wq + bq ; k = img_f @ Wk + bk ; v = img_f @ Wv + bv
  out = softmax(q k^T / sqrt(C)) v

Sharding: core i = (batch b = i//2, query-half h = i%2). Each core computes
q^T for its 1024 query rows and the FULL k^T / v for its batch (the host
ships the full img^T to both cores of a pair). Recomputing the peer's k/v
half costs ~55us of extra tensor time but removes the pairwise AllGather,
which measured 104us on the wire plus a 17us barrier and left the PE
clock-gate cold (the HAM re-throttles after ~3.4us of idle).

Layout strategy (zero on-device transposes):
  Host ships event^T / img^T (feature-major) and Wq/Wk/Wv natural, all fp16.
  - q^T[c,nq]  = (Wq blk).T @ ev^T     (lhsT = Wq, rhs = ev^T)
  - k^T[c,nk]  = (Wk blk).T @ img^T
  - v[nk,c]    = (img^T blk).T @ Wv    (lhsT = img^T, rhs = Wv)
  - s^T[nk,nq] = (k^T blk).T @ q^T     (scores transposed: k on partitions)
  - p^T = exp(s^T * scale)             (no max-subtraction; logits are O(5))
  - out[nq,c]  = (p^T blk).T @ v       (p^T is the stationary operand)
  - sums[nq,1] = (p^T blk).T @ ones    fused into the PV loop so the ones
    matmul shares the stationary p^T block (a separate sums pass measured
    ~170ns/matmul for 128 matmuls of one column each)
  - out *= 1/sums                      (normalize at the end)
All matmul operands fp16, PSUM accumulation fp32, output fp32.
"""

import json

import numpy as np

B, N, C = 4, 2048, 1024
NQ = N // 2          # query rows per core
CT = C // 128        # contraction tiles
KT = N // 128        # key-row tiles (full 2048 keys per core)
SCALE = 1.0 / np.sqrt(C)

_CACHE = {}


# ---------------------------------------------------------------------------
# Walrus in this container rejects >1 embedded sem-wait per instruction
# ("Too many sync wait commands"). Standalone waits are legal as
# EventSemaphore instructions, so hoist all but the last embedded wait.
def _fix_bir(bir: dict) -> dict:
    counter = [0]
    for fn in bir.get("functions", []):
        for bb in fn.get("blocks", []):
            out = []
            for ins in bb.get("instructions", []):
                si = ins.get("sync_info") or {}
                waits = si.get("on_wait") or []
                if len(waits) > 1 and ins.get("engine") not in (None, "Unassigned"):
                    for w in waits[:-1]:
                        counter[0] += 1
                        ev = {
                            "engine": ins["engine"],
                            "ins": [],
                            "name": f"hoistwait_{counter[0]}",
                            "opcode": "EventSemaphore",
                            "outs": [],
                            "sync_info": {"on_update": [], "on_wait": [w]},
                        }
                        if "debug" in ins:
                            ev["debug"] = ins["debug"]
                        out.append(ev)
                    si["on_wait"] = [waits[-1]]
                out.append(ins)
            bb["instructions"] = out
    return bir


def _install_waitfix(nc):
    orig = nc.to_json_bytes

    def patched():
        return json.dumps(_fix_bir(json.loads(orig()))).encode()

    nc.to_json_bytes = patched


# ---------------------------------------------------------------------------
def _build():
    import concourse.bass as bass
    import concourse.tile as tile
    from concourse import mybir

    f16, f32 = mybir.dt.float16, mybir.dt.float32
    Exp = mybir.ActivationFunctionType.Exp
    Ident = mybir.ActivationFunctionType.Identity

    nc = bass.Bass()
    ev_t = nc.dram_tensor("ev_t", [C, NQ], f16, kind="ExternalInput")
    img_t = nc.dram_tensor("img_t", [C, N], f16, kind="ExternalInput")
    wq = nc.dram_tensor("wq", [C, C], f16, kind="ExternalInput")
    wk = nc.dram_tensor("wk", [C, C], f16, kind="ExternalInput")
    wv = nc.dram_tensor("wv", [C, C], f16, kind="ExternalInput")
    bq = nc.dram_tensor("bq", [C], f32, kind="ExternalInput")
    bk = nc.dram_tensor("bk", [C], f32, kind="ExternalInput")
    bv = nc.dram_tensor("bv", [C], f32, kind="ExternalInput")
    out = nc.dram_tensor("out", [NQ, C], f32, kind="ExternalOutput")

    NCH = N // 512       # img column chunks
    with tile.TileContext(nc) as tc:
        with (
            tc.tile_pool(name="ins", bufs=1) as ins_pool,
            tc.tile_pool(name="qkv", bufs=1) as qkv_pool,
            tc.tile_pool(name="expp", bufs=1) as exp_pool,
            tc.tile_pool(name="work", bufs=2) as work,
            tc.tile_pool(name="ps_a", bufs=2, space="PSUM") as ps_a,
            tc.tile_pool(name="ps_b", bufs=2, space="PSUM") as ps_b,
            tc.tile_pool(name="ps_sc", bufs=2, space="PSUM") as ps_sc,
            tc.tile_pool(name="ps_sum", bufs=2, space="PSUM") as ps_sum,
        ):
            # ---- stage A: inputs to SBUF --------------------------------
            # A dma_start costs ~0.6us of ENGINE time on the issuing engine,
            # so keep DMAs few and large, and keep bulk issues off engines
            # that have early compute work. Sync carries the critical path
            # (biases first - a late bk stalls the k-proj ACT pipeline -
            # then wk + img in consumption order); Scalar carries the three
            # late-needed tensors (wv/wq/ev) and is done issuing by ~11us,
            # well before its first k-proj ACTIVATE at ~19us.
            wq_r = wq.rearrange("(t p) n -> p t n", p=128)
            wk_r = wk.rearrange("(t p) (h n) -> h p t n", p=128, n=512)
            wv_r = wv.rearrange("(t p) n -> p t n", p=128)
            ev_r = ev_t.rearrange("(t p) n -> p t n", p=128)
            img_r = img_t.rearrange("(t p) (c n) -> c p t n", p=128, n=512)

            bq_sb = ins_pool.tile([128, CT], f32)
            bk_sb = ins_pool.tile([128, CT], f32)
            nc.sync.dma_start(out=bq_sb[:], in_=bq.rearrange("(t p) -> p t", p=128))
            nc.sync.dma_start(out=bk_sb[:], in_=bk.rearrange("(t p) -> p t", p=128))
            # v-bias varies along the free dim -> broadcast row to 128 parts
            bv_sb = ins_pool.tile([128, C], f32)
            nc.sync.dma_start(out=bv_sb[:], in_=bv[None, :].to_broadcast((128, C)))

            # interleaved so the first k-proj tile (wk half 0 + img chunk 0)
            # unblocks as early as possible
            wk_sb = [
                ins_pool.tile([128, CT, 512], f16, name=f"wk{h}", tag=f"wk{h}")
                for h in range(2)
            ]
            img_sb = [
                ins_pool.tile([128, CT, 512], f16, name=f"img{ch}", tag=f"img{ch}")
                for ch in range(NCH)
            ]
            nc.sync.dma_start(out=wk_sb[0][:], in_=wk_r[0])
            nc.sync.dma_start(out=img_sb[0][:], in_=img_r[0])
            nc.sync.dma_start(out=wk_sb[1][:], in_=wk_r[1])
            for ch in range(1, NCH):
                nc.sync.dma_start(out=img_sb[ch][:], in_=img_r[ch])

            wv_sb = ins_pool.tile([128, CT, C], f16)
            nc.scalar.dma_start(out=wv_sb[:], in_=wv_r)
            wq_sb = ins_pool.tile([128, CT, C], f16)
            nc.scalar.dma_start(out=wq_sb[:], in_=wq_r)
            ev_sb = ins_pool.tile([128, CT, NQ], f16)
            nc.scalar.dma_start(out=ev_sb[:], in_=ev_r)

            ones_sb = ins_pool.tile([128, 1], f16)
            nc.vector.memset(ones_sb[:], 1.0)

            # ---- PE warmup ----------------------------------------------
            # The HAM clock gate holds the PE at 1.2GHz until it has been
            # busy ~3.4us, and re-throttles after ~3.4us idle. N=512 dummy
            # matmuls (N=128 ones have too low a duty cycle to trip the
            # busy-window) span the ~16us input-DMA window so real matmuls
            # start at 2.4GHz with no cold ramp.
            warm_sb = ins_pool.tile([128, 512], f16)
            nc.vector.memset(warm_sb[:], 0.0)
            for w in range(40):
                ps = ps_sc.tile([128, 512], f32, tag="sc")
                nc.tensor.matmul(ps, warm_sb[:, 0:128], warm_sb[:],
                                 start=True, stop=True)

            # ---- stage B: projections -----------------------------------
            # k^T over all 2048 keys; outer loop over img column chunks so
            # the first PSUM tile only needs wk + img chunk 0.
            k_sb = qkv_pool.tile([128, CT, N], f16)    # k^T  [c | nk]
            for ch in range(NCH):
                for co in range(CT):
                    ps = ps_a.tile([128, 512], f32, tag="a")
                    for t in range(CT):
                        nc.tensor.matmul(
                            ps,
                            wk_sb[co // 4][:, t, (co % 4) * 128:(co % 4 + 1) * 128],
                            img_sb[ch][:, t, :],
                            start=(t == 0), stop=(t == CT - 1),
                        )
                    nc.scalar.activation(
                        k_sb[:, co, ch * 512:(ch + 1) * 512], ps, Ident,
                        bias=bk_sb[:, co:co + 1],
                    )
            # v over all 2048 key rows
            v_sb = qkv_pool.tile([128, KT, C], f16)    # v    [nk | c]
            for nr in range(KT):
                ch, sub = nr // 4, nr % 4
                for cch in range(C // 512):
                    ps = ps_a.tile([128, 512], f32, tag="a")
                    for t in range(CT):
                        nc.tensor.matmul(
                            ps,
                            img_sb[ch][:, t, sub * 128:(sub + 1) * 128],
                            wv_sb[:, t, cch * 512:(cch + 1) * 512],
                            start=(t == 0), stop=(t == CT - 1),
                        )
                    nc.vector.tensor_add(
                        v_sb[:, nr, cch * 512:(cch + 1) * 512], ps,
                        bv_sb[:, cch * 512:(cch + 1) * 512],
                    )
            # q^T for this core's 1024 query rows
            q_sb = qkv_pool.tile([128, CT, NQ], f16)   # q^T  [c | nq]
            for co in range(CT):
                for ch in range(NQ // 512):
                    ps = ps_a.tile([128, 512], f32, tag="a")
                    for t in range(CT):
                        nc.tensor.matmul(
                            ps,
                            wq_sb[:, t, co * 128:(co + 1) * 128],
                            ev_sb[:, t, ch * 512:(ch + 1) * 512],
                            start=(t == 0), stop=(t == CT - 1),
                        )
                    nc.scalar.activation(
                        q_sb[:, co, ch * 512:(ch + 1) * 512], ps, Ident,
                        bias=bq_sb[:, co:co + 1],
                    )

            # ---- stage C: attention, 512 query rows at a time -----------
            for qc in range(NQ // 512):
                exp_sb = exp_pool.tile([128, KT, 512], f16, tag="exp")
                for kt in range(KT):
                    ps = ps_sc.tile([128, 512], f32, tag="sc")
                    for t in range(CT):
                        nc.tensor.matmul(
                            ps,
                            k_sb[:, t, kt * 128:(kt + 1) * 128],
                            q_sb[:, t, qc * 512:(qc + 1) * 512],
                            start=(t == 0), stop=(t == CT - 1),
                        )
                    nc.scalar.activation(exp_sb[:, kt, :], ps, Exp, scale=float(SCALE))

                for q4 in range(4):
                    qlo = q4 * 128
                    ps0 = ps_a.tile([128, 512], f32, tag="a")
                    ps1 = ps_b.tile([128, 512], f32, tag="b")
                    pss = ps_sum.tile([128, 1], f32, tag="s")
                    for kt in range(KT):
                        st, sp = (kt == 0), (kt == KT - 1)
                        p_blk = exp_sb[:, kt, qlo:qlo + 128]
                        nc.tensor.matmul(ps0, p_blk, v_sb[:, kt, 0:512],
                                         start=st, stop=sp)
                        nc.tensor.matmul(ps1, p_blk, v_sb[:, kt, 512:1024],
                                         start=st, stop=sp)
                        nc.tensor.matmul(pss, p_blk, ones_sb[:],
                                         start=st, stop=sp)
                    recip = work.tile([128, 1], f32, tag="recip")
                    nc.vector.reciprocal(recip[:], pss)
                    for cch, psv in ((0, ps0), (1, ps1)):
                        o_sb = work.tile([128, 512], f32, tag=f"o{cch}")
                        nc.vector.tensor_scalar_mul(o_sb[:], psv, recip[:])
                        nc.sync.dma_start(
                            out=out[qc * 512 + qlo:qc * 512 + qlo + 128,
                                    cch * 512:(cch + 1) * 512],
                            in_=o_sb[:],
                        )
    _install_waitfix(nc)
    return nc


def _get_nc():
    if "nc" not in _CACHE:
        _CACHE["nc"] = _build()
    return _CACHE["nc"]


def run(inputs, trace=False, trace_cores=None):
    from concourse.bass_utils import run_bass_kernel_spmd

    event_f = np.asarray(inputs["event_f"], dtype=np.float32)
    img_f = np.asarray(inputs["img_f"], dtype=np.float32)
    Wq = np.asarray(inputs["Wq"], dtype=np.float32).astype(np.float16)
    Wk = np.asarray(inputs["Wk"], dtype=np.float32).astype(np.float16)
    Wv = np.asarray(inputs["Wv"], dtype=np.float32).astype(np.float16)
    bq = np.asarray(inputs["bq"], dtype=np.float32)
    bk = np.asarray(inputs["bk"], dtype=np.float32)
    bv = np.asarray(inputs["bv"], dtype=np.float32)

    img_t_full = [
        np.ascontiguousarray(img_f[b].T.astype(np.float16)) for b in range(B)
    ]
    in_maps = []
    for core in range(8):
        b, h = core // 2, core % 2
        ev_t = np.ascontiguousarray(
            event_f[b, h * NQ:(h + 1) * NQ, :].T.astype(np.float16))
        in_maps.append({
            "ev_t": ev_t, "img_t": img_t_full[b],
            "wq": Wq, "wk": Wk, "wv": Wv,
            "bq": bq, "bk": bk, "bv": bv,
        })

    nc = _get_nc()
    res = run_bass_kernel_spmd(
        nc, in_maps, list(range(8)), trace=trace,
        **({"trace_cores": trace_cores} if trace_cores else {}),
    )
    full = np.empty((B, N, C), dtype=np.float32)
    for core in range(8):
        b, h = core // 2, core % 2
        full[b, h * NQ:(h + 1) * NQ, :] = res.results[core]["out"]
    return full, res


def kernel(**inputs) -> np.ndarray:
    full, _ = run(inputs, trace=False)
    return full


# revision 16
# speedup vs baseline: 1.2305x; 1.2305x over previous
"""Cross-attention kernel for Trainium2, SPMD over 8 NeuronCores.

Problem: B=4, N=2048, C=1024 fp32.
  q = event_f @ Wq + bq ; k = img_f @ Wk + bk ; v = img_f @ Wv + bv
  out = softmax(q k^T / sqrt(C)) v

Sharding: core i = (batch b = i//2, query-half h = i%2). Each core computes
q^T for its 1024 query rows and the FULL k^T / v for its batch (the host
ships the full img^T to both cores of a pair). Recomputing the peer's k/v
half costs ~55us of extra tensor time but removes the pairwise AllGather,
which measured 104us on the wire plus a 17us barrier and left the PE
clock-gate cold (the HAM re-throttles after ~3.4us of idle).

Layout strategy (zero on-device transposes):
  Host ships event^T / img^T (feature-major) and Wq/Wk/Wv natural, all fp16.
  - q^T[c,nq]  = (Wq blk).T @ ev^T     (lhsT = Wq, rhs = ev^T)
  - k^T[c,nk]  = (Wk blk).T @ img^T
  - v[nk,c]    = (img^T blk).T @ Wv    (lhsT = img^T, rhs = Wv)
  - s^T[nk,nq] = (k^T blk).T @ q^T     (scores transposed: k on partitions)
  - p^T = exp(s^T * scale)             (no max-subtraction; logits are O(5))
  - out[nq,c]  = (p^T blk).T @ v       (p^T is the stationary operand)
  - sums[nq,1] = (p^T blk).T @ ones    fused into the PV loop so the ones
    matmul shares the stationary p^T block (a separate sums pass measured
    ~170ns/matmul for 128 matmuls of one column each)
  - out *= 1/sums                      (normalize at the end)
All matmul operands fp16, PSUM accumulation fp32, output fp32.
"""

import json

import numpy as np

B, N, C = 4, 2048, 1024
NQ = N // 2          # query rows per core
CT = C // 128        # contraction tiles
KT = N // 128        # key-row tiles (full 2048 keys per core)
SCALE = 1.0 / np.sqrt(C)

_CACHE = {}


# ---------------------------------------------------------------------------
# Walrus in this container rejects >1 embedded sem-wait per instruction
# ("Too many sync wait commands"). Standalone waits are legal as
# EventSemaphore instructions, so hoist all but the last embedded wait.
def _fix_bir(bir: dict) -> dict:
    counter = [0]
    for fn in bir.get("functions", []):
        for bb in fn.get("blocks", []):
            out = []
            for ins in bb.get("instructions", []):
                si = ins.get("sync_info") or {}
                waits = si.get("on_wait") or []
                if len(waits) > 1 and ins.get("engine") not in (None, "Unassigned"):
                    for w in waits[:-1]:
                        counter[0] += 1
                        ev = {
                            "engine": ins["engine"],
                            "ins": [],
                            "name": f"hoistwait_{counter[0]}",
                            "opcode": "EventSemaphore",
                            "outs": [],
                            "sync_info": {"on_update": [], "on_wait": [w]},
                        }
                        if "debug" in ins:
                            ev["debug"] = ins["debug"]
                        out.append(ev)
                    si["on_wait"] = [waits[-1]]
                out.append(ins)
            bb["instructions"] = out
    return bir


def _install_waitfix(nc):
    orig = nc.to_json_bytes

    def patched():
        return json.dumps(_fix_bir(json.loads(orig()))).encode()

    nc.to_json_bytes = patched


# ---------------------------------------------------------------------------
def _build():
    import concourse.bass as bass
    import concourse.tile as tile
    from concourse import mybir

    f16, f32 = mybir.dt.float16, mybir.dt.float32
    Exp = mybir.ActivationFunctionType.Exp
    Ident = mybir.ActivationFunctionType.Identity

    nc = bass.Bass()
    ev_t = nc.dram_tensor("ev_t", [C, NQ], f16, kind="ExternalInput")
    img_t = nc.dram_tensor("img_t", [C, N], f16, kind="ExternalInput")
    wq = nc.dram_tensor("wq", [C, C], f16, kind="ExternalInput")
    wk = nc.dram_tensor("wk", [C, C], f16, kind="ExternalInput")
    wv = nc.dram_tensor("wv", [C, C], f16, kind="ExternalInput")
    # bqt/bkt are host-pre-transposed to the SBUF layout [128, CT] so the
    # load is one contiguous-row DMA; the natural "(t p) -> p t" scatter
    # costs ~2000 4-byte descriptors and chokes the queue head for ~19us.
    bqt = nc.dram_tensor("bqt", [128, CT], f32, kind="ExternalInput")
    bkt = nc.dram_tensor("bkt", [128, CT], f32, kind="ExternalInput")
    bv = nc.dram_tensor("bv", [C], f32, kind="ExternalInput")
    out = nc.dram_tensor("out", [NQ, C], f32, kind="ExternalOutput")

    NCH = N // 512       # img column chunks
    with tile.TileContext(nc) as tc:
        with (
            tc.tile_pool(name="ins", bufs=1) as ins_pool,
            tc.tile_pool(name="qkv", bufs=1) as qkv_pool,
            tc.tile_pool(name="expp", bufs=1) as exp_pool,
            tc.tile_pool(name="work", bufs=2) as work,
            tc.tile_pool(name="ps_a", bufs=2, space="PSUM") as ps_a,
            tc.tile_pool(name="ps_b", bufs=2, space="PSUM") as ps_b,
            tc.tile_pool(name="ps_sc", bufs=2, space="PSUM") as ps_sc,
            tc.tile_pool(name="ps_sum", bufs=2, space="PSUM") as ps_sum,
        ):
            # ---- stage A: inputs to SBUF --------------------------------
            # A dma_start costs ~0.6us of ENGINE time on the issuing engine,
            # so keep DMAs few and large, and keep bulk issues off engines
            # that have early compute work. Sync carries the critical path
            # (biases first - a late bk stalls the k-proj ACT pipeline -
            # then wk + img in consumption order); Scalar carries the three
            # late-needed tensors (wv/wq/ev) and is done issuing by ~11us,
            # well before its first k-proj ACTIVATE at ~19us.
            wq_r = wq.rearrange("(t p) n -> p t n", p=128)
            wk_r = wk.rearrange("(t p) (h n) -> h p t n", p=128, n=512)
            wv_r = wv.rearrange("(t p) n -> p t n", p=128)
            ev_r = ev_t.rearrange("(t p) n -> p t n", p=128)
            img_r = img_t.rearrange("(t p) (c n) -> c p t n", p=128, n=512)

            bq_sb = ins_pool.tile([128, CT], f32)
            bk_sb = ins_pool.tile([128, CT], f32)
            nc.scalar.dma_start(out=bk_sb[:], in_=bkt[:, :])
            nc.scalar.dma_start(out=bq_sb[:], in_=bqt[:, :])
            # v-bias varies along the free dim -> broadcast row to 128 parts
            bv_sb = ins_pool.tile([128, C], f32)
            nc.scalar.dma_start(out=bv_sb[:], in_=bv[None, :].to_broadcast((128, C)))

            # interleaved so the first k-proj tile (wk half 0 + img chunk 0)
            # unblocks as early as possible
            wk_sb = [
                ins_pool.tile([128, CT, 512], f16, name=f"wk{h}", tag=f"wk{h}")
                for h in range(2)
            ]
            img_sb = [
                ins_pool.tile([128, CT, 512], f16, name=f"img{ch}", tag=f"img{ch}")
                for ch in range(NCH)
            ]
            nc.sync.dma_start(out=wk_sb[0][:], in_=wk_r[0])
            nc.sync.dma_start(out=img_sb[0][:], in_=img_r[0])
            nc.sync.dma_start(out=wk_sb[1][:], in_=wk_r[1])
            for ch in range(1, NCH):
                nc.sync.dma_start(out=img_sb[ch][:], in_=img_r[ch])

            wv_sb = ins_pool.tile([128, CT, C], f16)
            nc.scalar.dma_start(out=wv_sb[:], in_=wv_r)
            wq_sb = ins_pool.tile([128, CT, C], f16)
            nc.scalar.dma_start(out=wq_sb[:], in_=wq_r)
            ev_sb = ins_pool.tile([128, CT, NQ], f16)
            nc.scalar.dma_start(out=ev_sb[:], in_=ev_r)

            ones_sb = ins_pool.tile([128, 1], f16)
            nc.vector.memset(ones_sb[:], 1.0)

            # ---- PE warmup ----------------------------------------------
            # The HAM clock gate holds the PE at 1.2GHz until it has been
            # busy ~3.4us, and re-throttles after ~3.4us idle. N=512 dummy
            # matmuls (N=128 ones have too low a duty cycle to trip the
            # busy-window) span the ~16us input-DMA window so real matmuls
            # start at 2.4GHz with no cold ramp.
            warm_sb = ins_pool.tile([128, 512], f16)
            nc.vector.memset(warm_sb[:], 0.0)
            for w in range(28):
                ps = ps_sc.tile([128, 512], f32, tag="sc")
                nc.tensor.matmul(ps, warm_sb[:, 0:128], warm_sb[:],
                                 start=True, stop=True)

            # ---- stage B: projections -----------------------------------
            # k^T over all 2048 keys; outer loop over img column chunks so
            # the first PSUM tile only needs wk + img chunk 0.
            k_sb = qkv_pool.tile([128, CT, N], f16)    # k^T  [c | nk]
            for ch in range(NCH):
                for co in range(CT):
                    ps = ps_a.tile([128, 512], f32, tag="a")
                    for t in range(CT):
                        nc.tensor.matmul(
                            ps,
                            wk_sb[co // 4][:, t, (co % 4) * 128:(co % 4 + 1) * 128],
                            img_sb[ch][:, t, :],
                            start=(t == 0), stop=(t == CT - 1),
                        )
                    nc.scalar.activation(
                        k_sb[:, co, ch * 512:(ch + 1) * 512], ps, Ident,
                        bias=bk_sb[:, co:co + 1],
                    )
            # v over all 2048 key rows
            v_sb = qkv_pool.tile([128, KT, C], f16)    # v    [nk | c]
            for nr in range(KT):
                ch, sub = nr // 4, nr % 4
                for cch in range(C // 512):
                    ps = ps_a.tile([128, 512], f32, tag="a")
                    for t in range(CT):
                        nc.tensor.matmul(
                            ps,
                            img_sb[ch][:, t, sub * 128:(sub + 1) * 128],
                            wv_sb[:, t, cch * 512:(cch + 1) * 512],
                            start=(t == 0), stop=(t == CT - 1),
                        )
                    nc.vector.tensor_add(
                        v_sb[:, nr, cch * 512:(cch + 1) * 512], ps,
                        bv_sb[:, cch * 512:(cch + 1) * 512],
                    )
            # q^T for this core's 1024 query rows
            q_sb = qkv_pool.tile([128, CT, NQ], f16)   # q^T  [c | nq]
            for co in range(CT):
                for ch in range(NQ // 512):
                    ps = ps_a.tile([128, 512], f32, tag="a")
                    for t in range(CT):
                        nc.tensor.matmul(
                            ps,
                            wq_sb[:, t, co * 128:(co + 1) * 128],
                            ev_sb[:, t, ch * 512:(ch + 1) * 512],
                            start=(t == 0), stop=(t == CT - 1),
                        )
                    nc.scalar.activation(
                        q_sb[:, co, ch * 512:(ch + 1) * 512], ps, Ident,
                        bias=bq_sb[:, co:co + 1],
                    )

            # ---- stage C: attention, 512 query rows at a time -----------
            for qc in range(NQ // 512):
                exp_sb = exp_pool.tile([128, KT, 512], f16, tag="exp")
                for kt in range(KT):
                    ps = ps_sc.tile([128, 512], f32, tag="sc")
                    for t in range(CT):
                        nc.tensor.matmul(
                            ps,
                            k_sb[:, t, kt * 128:(kt + 1) * 128],
                            q_sb[:, t, qc * 512:(qc + 1) * 512],
                            start=(t == 0), stop=(t == CT - 1),
                        )
                    nc.scalar.activation(exp_sb[:, kt, :], ps, Exp, scale=float(SCALE))

                for q4 in range(4):
                    qlo = q4 * 128
                    ps0 = ps_a.tile([128, 512], f32, tag="a")
                    ps1 = ps_b.tile([128, 512], f32, tag="b")
                    pss = ps_sum.tile([128, 1], f32, tag="s")
                    for kt in range(KT):
                        st, sp = (kt == 0), (kt == KT - 1)
                        p_blk = exp_sb[:, kt, qlo:qlo + 128]
                        nc.tensor.matmul(ps0, p_blk, v_sb[:, kt, 0:512],
                                         start=st, stop=sp)
                        nc.tensor.matmul(ps1, p_blk, v_sb[:, kt, 512:1024],
                                         start=st, stop=sp)
                        nc.tensor.matmul(pss, p_blk, ones_sb[:],
                                         start=st, stop=sp)
                    recip = work.tile([128, 1], f32, tag="recip")
                    nc.vector.reciprocal(recip[:], pss)
                    for cch, psv in ((0, ps0), (1, ps1)):
                        o_sb = work.tile([128, 512], f32, tag=f"o{cch}")
                        nc.vector.tensor_scalar_mul(o_sb[:], psv, recip[:])
                        nc.sync.dma_start(
                            out=out[qc * 512 + qlo:qc * 512 + qlo + 128,
                                    cch * 512:(cch + 1) * 512],
                            in_=o_sb[:],
                        )
    _install_waitfix(nc)
    return nc


def _get_nc():
    if "nc" not in _CACHE:
        _CACHE["nc"] = _build()
    return _CACHE["nc"]


def run(inputs, trace=False, trace_cores=None):
    from concourse.bass_utils import run_bass_kernel_spmd

    event_f = np.asarray(inputs["event_f"], dtype=np.float32)
    img_f = np.asarray(inputs["img_f"], dtype=np.float32)
    Wq = np.asarray(inputs["Wq"], dtype=np.float32).astype(np.float16)
    Wk = np.asarray(inputs["Wk"], dtype=np.float32).astype(np.float16)
    Wv = np.asarray(inputs["Wv"], dtype=np.float32).astype(np.float16)
    bq = np.asarray(inputs["bq"], dtype=np.float32)
    bk = np.asarray(inputs["bk"], dtype=np.float32)
    bv = np.asarray(inputs["bv"], dtype=np.float32)
    bqt = np.ascontiguousarray(bq.reshape(CT, 128).T)
    bkt = np.ascontiguousarray(bk.reshape(CT, 128).T)

    img_t_full = [
        np.ascontiguousarray(img_f[b].T.astype(np.float16)) for b in range(B)
    ]
    in_maps = []
    for core in range(8):
        b, h = core // 2, core % 2
        ev_t = np.ascontiguousarray(
            event_f[b, h * NQ:(h + 1) * NQ, :].T.astype(np.float16))
        in_maps.append({
            "ev_t": ev_t, "img_t": img_t_full[b],
            "wq": Wq, "wk": Wk, "wv": Wv,
            "bqt": bqt, "bkt": bkt, "bv": bv,
        })

    nc = _get_nc()
    res = run_bass_kernel_spmd(
        nc, in_maps, list(range(8)), trace=trace,
        **({"trace_cores": trace_cores} if trace_cores else {}),
    )
    full = np.empty((B, N, C), dtype=np.float32)
    for core in range(8):
        b, h = core // 2, core % 2
        full[b, h * NQ:(h + 1) * NQ, :] = res.results[core]["out"]
    return full, res


def kernel(**inputs) -> np.ndarray:
    full, _ = run(inputs, trace=False)
    return full


# revision 17
# speedup vs baseline: 1.5848x; 1.2879x over previous
"""Cross-attention kernel for Trainium2, SPMD over 8 NeuronCores.

Problem: B=4, N=2048, C=1024 fp32.
  q = event_f @ Wq + bq ; k = img_f @ Wk + bk ; v = img_f @ Wv + bv
  out = softmax(q k^T / sqrt(C)) v

Sharding: core i = (batch b = i//2, query-half h = i%2). Each core computes
q^T for its 1024 query rows and the FULL k^T / v for its batch (the host
ships the full img^T to both cores of a pair). Recomputing the peer's k/v
half costs ~55us of extra tensor time but removes the pairwise AllGather,
which measured 104us on the wire plus a 17us barrier and left the PE
clock-gate cold (the HAM re-throttles after ~3.4us of idle).

Layout strategy (zero on-device transposes):
  Host ships event^T / img^T (feature-major) and Wq/Wk/Wv natural, all fp16.
  - q^T[c,nq]  = (Wq blk).T @ ev^T     (lhsT = Wq, rhs = ev^T)
  - k^T[c,nk]  = (Wk blk).T @ img^T
  - v[nk,c]    = (img^T blk).T @ Wv    (lhsT = img^T, rhs = Wv)
  - s^T[nk,nq] = (k^T blk).T @ q^T     (scores transposed: k on partitions)
  - p^T = exp(s^T * scale)             (no max-subtraction; logits are O(5))
  - out[nq,c]  = (p^T blk).T @ v       (p^T is the stationary operand)
  - sums[nq,1] = (p^T blk).T @ ones    fused into the PV loop so the ones
    matmul shares the stationary p^T block (a separate sums pass measured
    ~170ns/matmul for 128 matmuls of one column each)
  - out *= 1/sums                      (normalize at the end)
All matmul operands fp16, PSUM accumulation fp32, output fp32.
"""

import json

import numpy as np

B, N, C = 4, 2048, 1024
NQ = N // 2          # query rows per core
CT = C // 128        # contraction tiles
KT = N // 128        # key-row tiles (full 2048 keys per core)
SCALE = 1.0 / np.sqrt(C)

_CACHE = {}


# ---------------------------------------------------------------------------
# Walrus in this container rejects >1 embedded sem-wait per instruction
# ("Too many sync wait commands"). Standalone waits are legal as
# EventSemaphore instructions, so hoist all but the last embedded wait.
def _fix_bir(bir: dict) -> dict:
    counter = [0]
    for fn in bir.get("functions", []):
        for bb in fn.get("blocks", []):
            out = []
            for ins in bb.get("instructions", []):
                si = ins.get("sync_info") or {}
                waits = si.get("on_wait") or []
                if len(waits) > 1 and ins.get("engine") not in (None, "Unassigned"):
                    for w in waits[:-1]:
                        counter[0] += 1
                        ev = {
                            "engine": ins["engine"],
                            "ins": [],
                            "name": f"hoistwait_{counter[0]}",
                            "opcode": "EventSemaphore",
                            "outs": [],
                            "sync_info": {"on_update": [], "on_wait": [w]},
                        }
                        if "debug" in ins:
                            ev["debug"] = ins["debug"]
                        out.append(ev)
                    si["on_wait"] = [waits[-1]]
                out.append(ins)
            bb["instructions"] = out
    return bir


def _install_waitfix(nc):
    orig = nc.to_json_bytes

    def patched():
        return json.dumps(_fix_bir(json.loads(orig()))).encode()

    nc.to_json_bytes = patched


# ---------------------------------------------------------------------------
def _build():
    import concourse.bass as bass
    import concourse.tile as tile
    from concourse import mybir

    f16, f32 = mybir.dt.float16, mybir.dt.float32
    Exp = mybir.ActivationFunctionType.Exp
    Ident = mybir.ActivationFunctionType.Identity

    nc = bass.Bass()
    ev_t = nc.dram_tensor("ev_t", [C, NQ], f16, kind="ExternalInput")
    img_t = nc.dram_tensor("img_t", [C, N], f16, kind="ExternalInput")
    wq = nc.dram_tensor("wq", [C, C], f16, kind="ExternalInput")
    wk = nc.dram_tensor("wk", [C, C], f16, kind="ExternalInput")
    wv = nc.dram_tensor("wv", [C, C], f16, kind="ExternalInput")
    # bqt/bkt are host-pre-transposed to the SBUF layout [128, CT] so the
    # load is one contiguous-row DMA; the natural "(t p) -> p t" scatter
    # costs ~2000 4-byte descriptors and chokes the queue head for ~19us.
    bqt = nc.dram_tensor("bqt", [128, CT], f32, kind="ExternalInput")
    bkt = nc.dram_tensor("bkt", [128, CT], f32, kind="ExternalInput")
    bv = nc.dram_tensor("bv", [C], f32, kind="ExternalInput")
    out = nc.dram_tensor("out", [NQ, C], f32, kind="ExternalOutput")

    NCH = N // 512       # img column chunks
    with tile.TileContext(nc) as tc:
        with (
            tc.tile_pool(name="ins", bufs=1) as ins_pool,
            tc.tile_pool(name="qkv", bufs=1) as qkv_pool,
            tc.tile_pool(name="expp", bufs=1) as exp_pool,
            tc.tile_pool(name="work", bufs=2) as work,
            tc.tile_pool(name="ps_a", bufs=2, space="PSUM") as ps_a,
            tc.tile_pool(name="ps_b", bufs=2, space="PSUM") as ps_b,
            tc.tile_pool(name="ps_sc", bufs=2, space="PSUM") as ps_sc,
            tc.tile_pool(name="ps_sum", bufs=2, space="PSUM") as ps_sum,
        ):
            # ---- stage A: inputs to SBUF --------------------------------
            # A dma_start costs ~0.6us of ENGINE time on the issuing engine,
            # so keep DMAs few and large, and keep bulk issues off engines
            # that have early compute work. Sync carries the critical path
            # (biases first - a late bk stalls the k-proj ACT pipeline -
            # then wk + img in consumption order); Scalar carries the three
            # late-needed tensors (wv/wq/ev) and is done issuing by ~11us,
            # well before its first k-proj ACTIVATE at ~19us.
            wq_r = wq.rearrange("(t p) n -> p t n", p=128)
            wk_r = wk.rearrange("(t p) (h n) -> h p t n", p=128, n=512)
            wv_r = wv.rearrange("(t p) n -> p t n", p=128)
            ev_r = ev_t.rearrange("(t p) n -> p t n", p=128)
            img_r = img_t.rearrange("(t p) (c n) -> c p t n", p=128, n=512)

            bq_sb = ins_pool.tile([128, CT], f32)
            bk_sb = ins_pool.tile([128, CT], f32)
            nc.scalar.dma_start(out=bk_sb[:], in_=bkt[:, :])
            nc.scalar.dma_start(out=bq_sb[:], in_=bqt[:, :])
            # v-bias varies along the free dim -> broadcast row to 128 parts
            bv_sb = ins_pool.tile([128, C], f32)
            nc.scalar.dma_start(out=bv_sb[:], in_=bv[None, :].to_broadcast((128, C)))

            # interleaved so the first k-proj tile (wk half 0 + img chunk 0)
            # unblocks as early as possible
            wk_sb = [
                ins_pool.tile([128, CT, 512], f16, name=f"wk{h}", tag=f"wk{h}")
                for h in range(2)
            ]
            img_sb = [
                ins_pool.tile([128, CT, 512], f16, name=f"img{ch}", tag=f"img{ch}")
                for ch in range(NCH)
            ]
            nc.sync.dma_start(out=wk_sb[0][:], in_=wk_r[0])
            nc.sync.dma_start(out=img_sb[0][:], in_=img_r[0])
            nc.sync.dma_start(out=wk_sb[1][:], in_=wk_r[1])
            for ch in range(1, NCH):
                nc.sync.dma_start(out=img_sb[ch][:], in_=img_r[ch])

            wv_sb = ins_pool.tile([128, CT, C], f16)
            nc.scalar.dma_start(out=wv_sb[:], in_=wv_r)
            wq_sb = ins_pool.tile([128, CT, C], f16)
            nc.scalar.dma_start(out=wq_sb[:], in_=wq_r)
            ev_sb = ins_pool.tile([128, CT, NQ], f16)
            nc.scalar.dma_start(out=ev_sb[:], in_=ev_r)

            ones_sb = ins_pool.tile([128, 1], f16)
            nc.vector.memset(ones_sb[:], 1.0)

            # ---- PE warmup ----------------------------------------------
            # The HAM clock gate holds the PE at 1.2GHz until it has been
            # busy ~3.4us, and re-throttles after ~3.4us idle. N=512 dummy
            # matmuls (N=128 ones have too low a duty cycle to trip the
            # busy-window) span the ~16us input-DMA window so real matmuls
            # start at 2.4GHz with no cold ramp.
            warm_sb = ins_pool.tile([128, 512], f16)
            nc.vector.memset(warm_sb[:], 0.0)
            for w in range(56):
                ps = ps_sc.tile([128, 512], f32, tag="sc")
                nc.tensor.matmul(ps, warm_sb[:, 0:128], warm_sb[:],
                                 start=True, stop=True)

            # ---- stage B: projections -----------------------------------
            # k^T over all 2048 keys; outer loop over img column chunks so
            # the first PSUM tile only needs wk + img chunk 0.
            k_sb = qkv_pool.tile([128, CT, N], f16)    # k^T  [c | nk]
            for ch in range(NCH):
                for co in range(CT):
                    ps = ps_a.tile([128, 512], f32, tag="a")
                    for t in range(CT):
                        nc.tensor.matmul(
                            ps,
                            wk_sb[co // 4][:, t, (co % 4) * 128:(co % 4 + 1) * 128],
                            img_sb[ch][:, t, :],
                            start=(t == 0), stop=(t == CT - 1),
                        )
                    nc.scalar.activation(
                        k_sb[:, co, ch * 512:(ch + 1) * 512], ps, Ident,
                        bias=bk_sb[:, co:co + 1],
                    )
            # v over all 2048 key rows
            v_sb = qkv_pool.tile([128, KT, C], f16)    # v    [nk | c]
            for nr in range(KT):
                ch, sub = nr // 4, nr % 4
                for cch in range(C // 512):
                    ps = ps_a.tile([128, 512], f32, tag="a")
                    for t in range(CT):
                        nc.tensor.matmul(
                            ps,
                            img_sb[ch][:, t, sub * 128:(sub + 1) * 128],
                            wv_sb[:, t, cch * 512:(cch + 1) * 512],
                            start=(t == 0), stop=(t == CT - 1),
                        )
                    nc.vector.tensor_add(
                        v_sb[:, nr, cch * 512:(cch + 1) * 512], ps,
                        bv_sb[:, cch * 512:(cch + 1) * 512],
                    )
            # q^T for this core's 1024 query rows
            q_sb = qkv_pool.tile([128, CT, NQ], f16)   # q^T  [c | nq]
            for co in range(CT):
                for ch in range(NQ // 512):
                    ps = ps_a.tile([128, 512], f32, tag="a")
                    for t in range(CT):
                        nc.tensor.matmul(
                            ps,
                            wq_sb[:, t, co * 128:(co + 1) * 128],
                            ev_sb[:, t, ch * 512:(ch + 1) * 512],
                            start=(t == 0), stop=(t == CT - 1),
                        )
                    nc.scalar.activation(
                        q_sb[:, co, ch * 512:(ch + 1) * 512], ps, Ident,
                        bias=bq_sb[:, co:co + 1],
                    )

            # ---- stage C: attention, 512 query rows at a time -----------
            for qc in range(NQ // 512):
                exp_sb = exp_pool.tile([128, KT, 512], f16, tag="exp")
                for kt in range(KT):
                    ps = ps_sc.tile([128, 512], f32, tag="sc")
                    for t in range(CT):
                        nc.tensor.matmul(
                            ps,
                            k_sb[:, t, kt * 128:(kt + 1) * 128],
                            q_sb[:, t, qc * 512:(qc + 1) * 512],
                            start=(t == 0), stop=(t == CT - 1),
                        )
                    nc.scalar.activation(exp_sb[:, kt, :], ps, Exp, scale=float(SCALE))

                for q4 in range(4):
                    qlo = q4 * 128
                    ps0 = ps_a.tile([128, 512], f32, tag="a")
                    ps1 = ps_b.tile([128, 512], f32, tag="b")
                    pss = ps_sum.tile([128, 1], f32, tag="s")
                    for kt in range(KT):
                        st, sp = (kt == 0), (kt == KT - 1)
                        p_blk = exp_sb[:, kt, qlo:qlo + 128]
                        nc.tensor.matmul(ps0, p_blk, v_sb[:, kt, 0:512],
                                         start=st, stop=sp)
                        nc.tensor.matmul(ps1, p_blk, v_sb[:, kt, 512:1024],
                                         start=st, stop=sp)
                        nc.tensor.matmul(pss, p_blk, ones_sb[:],
                                         start=st, stop=sp)
                    recip = work.tile([128, 1], f32, tag="recip")
                    nc.vector.reciprocal(recip[:], pss)
                    for cch, psv in ((0, ps0), (1, ps1)):
                        o_sb = work.tile([128, 512], f32, tag=f"o{cch}")
                        nc.vector.tensor_scalar_mul(o_sb[:], psv, recip[:])
                        nc.sync.dma_start(
                            out=out[qc * 512 + qlo:qc * 512 + qlo + 128,
                                    cch * 512:(cch + 1) * 512],
                            in_=o_sb[:],
                        )
    _install_waitfix(nc)
    return nc


def _get_nc():
    if "nc" not in _CACHE:
        _CACHE["nc"] = _build()
    return _CACHE["nc"]


def run(inputs, trace=False, trace_cores=None):
    from concourse.bass_utils import run_bass_kernel_spmd

    event_f = np.asarray(inputs["event_f"], dtype=np.float32)
    img_f = np.asarray(inputs["img_f"], dtype=np.float32)
    Wq = np.asarray(inputs["Wq"], dtype=np.float32).astype(np.float16)
    Wk = np.asarray(inputs["Wk"], dtype=np.float32).astype(np.float16)
    Wv = np.asarray(inputs["Wv"], dtype=np.float32).astype(np.float16)
    bq = np.asarray(inputs["bq"], dtype=np.float32)
    bk = np.asarray(inputs["bk"], dtype=np.float32)
    bv = np.asarray(inputs["bv"], dtype=np.float32)
    bqt = np.ascontiguousarray(bq.reshape(CT, 128).T)
    bkt = np.ascontiguousarray(bk.reshape(CT, 128).T)

    img_t_full = [
        np.ascontiguousarray(img_f[b].T.astype(np.float16)) for b in range(B)
    ]
    in_maps = []
    for core in range(8):
        b, h = core // 2, core % 2
        ev_t = np.ascontiguousarray(
            event_f[b, h * NQ:(h + 1) * NQ, :].T.astype(np.float16))
        in_maps.append({
            "ev_t": ev_t, "img_t": img_t_full[b],
            "wq": Wq, "wk": Wk, "wv": Wv,
            "bqt": bqt, "bkt": bkt, "bv": bv,
        })

    nc = _get_nc()
    res = run_bass_kernel_spmd(
        nc, in_maps, list(range(8)), trace=trace,
        **({"trace_cores": trace_cores} if trace_cores else {}),
    )
    full = np.empty((B, N, C), dtype=np.float32)
    for core in range(8):
        b, h = core // 2, core % 2
        full[b, h * NQ:(h + 1) * NQ, :] = res.results[core]["out"]
    return full, res


def kernel(**inputs) -> np.ndarray:
    full, _ = run(inputs, trace=False)
    return full
